# revision 7
# baseline (speedup 1.0000x reference)
"""Chamfer distance kernel for Trainium2 (8 NeuronCores).

Problem: input1 [4,8192,3], input2 [4,8192,3] f32.
  d2[b,n,m] = ||x_bn - y_bm||^2 (clamped at 0)
  out = mean_n(min_m d2) + mean_m(min_n d2)   (scalar f32)

Sharding: 8 cores = 4 batches x 2 halves of N. Each core computes its
4096x8192 block of the distance matrix once.

The whole pipeline works on NEGATED distances so every min becomes a
max (gpsimd.partition_all_reduce only supports max): the K=20 fp16
"double-double" augmented matmul  -d2[n,m] = [-x2,-1,-x].[1,y2,-2y]
accumulates -d2 in fp32 PSUM at ~1e-5 abs accuracy and full PE rate.

Per 128-row tile (2 tiles per dynamic-loop iteration, double-buffered):
the m sweep runs in two 4096-wide halves through all 8 PSUM banks; ACT
evacuates each half to SBUF as fp16 (2x2048 copies per half so the next
half's matmuls can start on freed banks). DVE then does one
tensor_tensor_reduce (row-max of the 8192-wide fp16 tile, accumulator
written straight into R1[:, t]) and one fp16 tensor_tensor col-max into
acc2[:, 0:CW]; the Pool engine (otherwise idle) handles the col-max of
the remaining acc2[:, CW:8192] slice. fp16 tensor_tensor runs the DVE
at 2 elem/cycle (2x_1p); the reduce pairs its two 4096-wide inputs so
both row and col passes cost ~0.5 cycle/element.

Finale: one gpsimd.partition_all_reduce(max) folds acc2 128->1 for
dist2. Host: negate, clamp at 0, merge per-core halves, mean.
"""

import os
import sys

import numpy as np

for _p in ("/opt/trn_rl_repo", "/root/.axon_site/_ro/trn_rl_repo"):
    if os.path.isdir(_p) and _p not in sys.path:
        sys.path.insert(0, _p)
        break

import concourse.bass as bass
import concourse.tile as tile
from concourse import mybir, bacc, bass_isa
from concourse.bass_utils import run_bass_kernel_spmd

B, N, M, D = 4, 8192, 8192, 3
NCORES = 8
HALF = N // 2
NEG_BIG = -60000.0  # fp16-safe "minus infinity" (|−d2| << 60000)

_prog_cache: dict = {}


def build_program(
    n_rows: int = HALF, m_cols: int = M, repeat: int = 1
) -> bass.Bass:
    """One-core program. Inputs: aug [20, n_rows+m_cols] fp16 =
    [stat20(-x) | mov20(y)]; outputs: out1 [n_rows] f32 (NEGATED row max
    = -min_m d2 per n-row), out2 [m_cols] f16 (negated col max)."""
    f16 = mybir.dt.float16
    f32 = mybir.dt.float32
    mx = mybir.AluOpType.max

    PH = 4096  # psum half-sweep width (all 8 banks)
    NT = n_rows // 128
    assert n_rows % 256 == 0 and m_cols == 2 * PH
    NI = NT // 2  # loop iterations, 2 tiles each

    nc = bacc.Bacc()
    W = n_rows + m_cols
    aug = nc.declare_dram_parameter("aug", [20, W], f16, isOutput=False)
    out1 = nc.declare_dram_parameter("out1", [n_rows], f32, isOutput=True)
    out2 = nc.declare_dram_parameter("out2", [m_cols], f16, isOutput=True)

    with tile.TileContext(nc) as tc:
        with (
            tc.tile_pool(name="consts", bufs=1) as consts,
            tc.tile_pool(name="psump", bufs=1, space="PSUM") as psump,
        ):
            aug_t = consts.tile([20, W], f16)
            nc.gpsimd.dma_start(out=aug_t, in_=aug[:, :])
            as_t = aug_t[:, 0:n_rows]
            bm_t = aug_t[:, n_rows:W]

            R1 = consts.tile([128, NT], f32)
            acc2 = consts.tile([128, m_cols], f16)
            allr = consts.tile([128, m_cols], f16)
            stat_buf = consts.tile([20, 256], f16)
            bufs = [consts.tile([128, m_cols], f16, name=f"bf{x}") for x in range(2)]
            scrap = consts.tile([128, PH], f16)
            ps_all = psump.tile([128, PH], f32)

            nc.vector.memset(acc2, NEG_BIG)

            def tile_body(x, t_expr):
                """One 128-row tile: x = buffer parity, t_expr = dynamic
                tile index expression (for R1 column)."""
                buf = bufs[x]
                lhs = stat_buf[:, x * 128 : (x + 1) * 128]
                for h in range(2):
                    for q in range(8):
                        col = h * PH + q * 512
                        nc.tensor.matmul(
                            ps_all[:, q * 512 : (q + 1) * 512],
                            lhsT=lhs,
                            rhs=bm_t[:, col : col + 512],
                            start=True,
                            stop=True,
                        )
                    for e in range(2):
                        nc.scalar.copy(
                            out=buf[:, h * PH + e * 2048 : h * PH + (e + 1) * 2048],
                            in_=ps_all[:, e * 2048 : (e + 1) * 2048],
                        )
                # row max (negated): one max-scan pairing the two tile
                # halves; last column = row max, copied into R1[:, t]
                nc.vector.tensor_tensor_scan(
                    out=scrap,
                    data0=buf[:, 0:PH],
                    data1=buf[:, PH : 2 * PH],
                    initial=NEG_BIG,
                    op0=mx,
                    op1=mx,
                )
                nc.vector.tensor_copy(
                    out=R1[:, t_expr], in_=scrap[:, PH - 1 : PH]
                )
                # col max: single fp16 tensor_tensor (2x_1p, 2 elem/cycle).
                # Pool cannot run TensorTensor (walrus codegen rejects it),
                # so DVE owns the full width.
                nc.vector.tensor_tensor(
                    out=acc2[:, 0:m_cols],
                    in0=buf[:, 0:m_cols],
                    in1=acc2[:, 0:m_cols],
                    op=mx,
                )

            def iter_body(i):
                # stationary slices for both tiles of this iteration
                nc.vector.tensor_copy(
                    out=stat_buf, in_=as_t[:, bass.ds(i * 256, 256)]
                )
                tile_body(0, bass.ds(i * 2, 1))
                tile_body(1, bass.ds(i * 2 + 1, 1))

            if repeat == 1:
                with tc.For_i(0, NI, 1) as i:
                    iter_body(i)
            else:
                with tc.For_i(0, repeat, 1) as _r:
                    with tc.For_i(0, NI, 1) as i:
                        iter_body(i)

            # dist2 finale: fold 128 partitions -> 1 via SBUF->SBUF DMA
            # shifts + fp16 elementwise max (proven-on-HW pattern)
            p = 64
            while p >= 1:
                nc.gpsimd.dma_start(out=allr[0:p, :], in_=acc2[p : 2 * p, :])
                nc.vector.tensor_tensor(
                    out=acc2[0:p, :], in0=allr[0:p, :], in1=acc2[0:p, :], op=mx
                )
                p //= 2

            nc.gpsimd.dma_start(out=out1[:].rearrange("(i p) -> p i", p=128), in_=R1)
            nc.gpsimd.dma_start(out=out2[:], in_=acc2[0:1, :])

    nc.finalize()
    return nc


def _get_program(n_rows: int, m_cols: int) -> bass.Bass:
    key = (n_rows, m_cols)
    if key not in _prog_cache:
        _prog_cache[key] = build_program(n_rows, m_cols)
    return _prog_cache[key]


def _aug(pts: np.ndarray):
    """pts [n,3] -> (negated stationary [5,n], moving [5,n]) augmented
    forms: (-stat).mov = -d2."""
    pts = np.asarray(pts, np.float32)
    sq = (pts * pts).sum(-1)
    ones = np.ones_like(sq)
    stat = np.ascontiguousarray(
        -np.stack([sq, ones, pts[:, 0], pts[:, 1], pts[:, 2]]), dtype=np.float32
    )
    movg = np.ascontiguousarray(
        np.stack([ones, sq, -2.0 * pts[:, 0], -2.0 * pts[:, 1], -2.0 * pts[:, 2]]),
        dtype=np.float32,
    )
    return stat, movg


def _split16(a: np.ndarray):
    hi = a.astype(np.float16)
    lo = (a.astype(np.float64) - hi.astype(np.float64)).astype(np.float16)
    return hi, lo


def pack_aug(x: np.ndarray, y: np.ndarray) -> np.ndarray:
    """fp16 double-double packing: [20, n+m] = [stat20(-x) | mov20(y)]."""
    a_s, _ = _aug(x)
    _, b_m = _aug(y)
    ah, al = _split16(a_s)
    bh, bl = _split16(b_m)
    stat20 = np.concatenate([ah, ah, al, al], axis=0)  # [20, n]
    mov20 = np.concatenate([bh, bl, bh, bl], axis=0)  # [20, m]
    return np.ascontiguousarray(
        np.concatenate([stat20, mov20], axis=1), dtype=np.float16
    )


def make_in_maps(input1: np.ndarray, input2: np.ndarray):
    in_maps = []
    for c in range(NCORES):
        b, h = divmod(c, 2)
        x = input1[b, h * HALF : (h + 1) * HALF]
        y = input2[b]
        in_maps.append({"aug": pack_aug(x, y)})
    return in_maps


def combine(results) -> np.ndarray:
    d1 = np.zeros((B, N), np.float32)
    nm2 = np.full((B, M), np.float32(NEG_BIG), np.float32)
    for c in range(NCORES):
        b, h = divmod(c, 2)
        d1[b, h * HALF : (h + 1) * HALF] = -np.asarray(results[c]["out1"], np.float32)
        nm2[b] = np.maximum(nm2[b], np.asarray(results[c]["out2"], np.float32))
    d1 = np.maximum(d1, 0.0)
    d2 = np.maximum(-nm2, 0.0)
    val = d1.mean(dtype=np.float64) + d2.mean(dtype=np.float64)
    return np.asarray(val, dtype=np.float32)


def run_on_hw(input1, input2, **kwargs):
    nc = _get_program(HALF, M)
    in_maps = make_in_maps(
        np.asarray(input1, np.float32), np.asarray(input2, np.float32)
    )
    return run_bass_kernel_spmd(nc, in_maps, list(range(NCORES)), **kwargs)


def kernel(input1: np.ndarray, input2: np.ndarray) -> np.ndarray:
    res = run_on_hw(input1, input2)
    return combine(res.results)
